# revision 4
# baseline (speedup 1.0000x reference)
"""Trainium2 Bass kernel for nn_CausalSelfAttention (B=1, T=2048, D=1024, H=16).

Sharding: 2 heads per core across 8 cores (tensor parallel). Wq/Wk/Wv
column-sharded by head, attention fully local, Wo row-sharded; host sums the
8 partial outputs (the all-reduce of the unshard step).

Per-core pipeline (all matmuls fp16 operands, fp32 PSUM accumulate):
  P1  fused QKV: psum[t,384] = sum_i xT_blk.T @ [WqT|WkT|(1-l)WvT]; wqkv DMA'd
      per-i so accumulation streams behind the loads. RMS stats (Square on ACT
      from PSUM + segmented reduce -> rsqrt bit-trick, 1 Newton iter, 0.125
      folded into k's scale), RoPE with the swap absorbed into strided views,
      v-blend (+lam*vi) via DVE scalar_tensor_tensor straight from PSUM,
      [vA|1|vB|1] fp16 tiles with ones columns memset once.
  P2  PE-transpose roped q,k (fp16 all the way, fp16 PSUM) -> qT,kT fp16.
  P3  attention software-pipelined per (head, kj) unit: score matmul into a
      1-bank PSUM tile, Exp (ACT) on just the live columns, tri-mask on
      diagonal blocks (DVE/Pool alternating), P@V matmul lagging 3 units so
      the PE never waits on Exp; lhsT=[v|1] accumulates L in row 64.
  P4  scale = 1/(L + e^sink): row-64 + e^sink (DVE) -> reciprocal ->
      partition-broadcast view into the yts scale mult; out-proj per 128-row
      block (K=128), evacuate fp16, DMA fp16 partials on alternating queues;
      host upcasts + sums.
"""

import sys

if "/opt/trn_rl_repo" not in sys.path:
    sys.path.insert(0, "/opt/trn_rl_repo")

import numpy as np
from contextlib import ExitStack

from concourse import bacc, tile
from concourse import mybir
from concourse.bass_utils import run_bass_kernel_spmd

F32 = mybir.dt.float32
F32R = mybir.dt.float32r
F16 = mybir.dt.float16
I32 = mybir.dt.int32
AF = mybir.ActivationFunctionType
ALU = mybir.AluOpType
AX = mybir.AxisListType

T = 2048
D = 1024
HD = 64
NT = T // 128  # 16 t-tiles
RMS_EPS = float(np.finfo(np.float32).eps)


def _build_program():
    nc = bacc.Bacc("TRN2", target_bir_lowering=False, debug=False, num_devices=8)

    d_xtb = nc.dram_tensor("xtb", [NT, 128, 8, 128], F16, kind="ExternalInput").ap()
    d_wqkv = nc.dram_tensor("wqkv", [128, 8, 384], F16, kind="ExternalInput").ap()
    d_vis = nc.dram_tensor("vis", [128, NT, 128], F16, kind="ExternalInput").ap()
    d_cc = nc.dram_tensor("cc", [128, NT, 64], F32, kind="ExternalInput").ap()
    d_sc = nc.dram_tensor("sc", [128, NT, 64], F32, kind="ExternalInput").ap()
    d_wo = nc.dram_tensor("wo", [128, D], F16, kind="ExternalInput").ap()
    d_idn = nc.dram_tensor("idn", [128, 128], F16, kind="ExternalInput").ap()
    d_tri = nc.dram_tensor("tri", [128, 128], F16, kind="ExternalInput").ap()
    d_esk = nc.dram_tensor("esk", [1, 2], F32, kind="ExternalInput").ap()
    d_out = nc.dram_tensor("out", [D, T], F16, kind="ExternalOutput").ap()

    with tile.TileContext(nc) as tc, ExitStack() as ctx:
        sb = ctx.enter_context(tc.tile_pool(name="sb", bufs=1))
        sb_x = ctx.enter_context(tc.tile_pool(name="sb_x", bufs=5))
        sb_w1 = ctx.enter_context(tc.tile_pool(name="sb_w1", bufs=3))
        sb_w2 = ctx.enter_context(tc.tile_pool(name="sb_w2", bufs=3))
        sb_e = ctx.enter_context(tc.tile_pool(name="sb_e", bufs=6))
        sb_o = ctx.enter_context(tc.tile_pool(name="sb_o", bufs=3))
        ps = ctx.enter_context(tc.tile_pool(name="ps", bufs=2, space="PSUM"))
        ps3 = ctx.enter_context(tc.tile_pool(name="ps3", bufs=3, space="PSUM"))

        # weights first on the sync queue (needed by the first matmul),
        # split per-i so QKV accumulation can stream behind the loads;
        # other constants go via the gpsimd queue so they don't delay x.
        wqkv = sb.tile([128, 8, 384], F16)
        wq_dmas = []
        for _wi in range(8):
            wq_dmas.append(
                nc.sync.dma_start(
                    out=wqkv[:, _wi, :], in_=d_wqkv[:, _wi, :]
                )
            )
        vi_t = sb.tile([128, NT, 128], F16)
        cc_t = sb.tile([128, NT, 64], F32)
        sc_t = sb.tile([128, NT, 64], F32)
        wo = sb.tile([128, D], F16)
        esk = sb.tile([1, 2], F32)
        const_dmas = []
        early_dmas = []
        early_dmas.append(nc.gpsimd.dma_start(out=vi_t[:], in_=d_vis[:]))
        early_dmas.append(nc.gpsimd.dma_start(out=cc_t[:], in_=d_cc[:]))
        early_dmas.append(nc.gpsimd.dma_start(out=sc_t[:], in_=d_sc[:]))
        early_dmas.append(nc.gpsimd.dma_start(out=esk[:], in_=d_esk[:]))
        const_dmas.append(nc.gpsimd.dma_start(out=wo[:], in_=d_wo[:]))
        idn = sb.tile([128, 128], F16)
        early_dmas.append(nc.gpsimd.dma_start(out=idn[:], in_=d_idn[:]))
        tri = sb.tile([128, 128], F16)
        const_dmas.append(nc.gpsimd.dma_start(out=tri[:], in_=d_tri[:]))

        stats = sb.tile([128, 64], F32)
        rbuf = sb.tile([128, 64], F32)
        qT = sb.tile([128, T], F16)
        kT = sb.tile([128, T], F16)
        vtiles = [sb.tile([128, 130], F16, tag=f"v{i}", name=f"v{i}") for i in range(NT)]
        qkr = [sb.tile([128, 256], F16, tag=f"qkr{i}", name=f"qkr{i}") for i in range(NT)]
        qkro = [sb.tile([128, 256], F32, tag=f"qkro{i}", name=f"qkro{i}") for i in range(NT)]
        yts = sb.tile([128, T], F16)

        # ones columns of the [vA|1|vB|1] tiles: written once, up front
        for i in range(NT):
            nc.gpsimd.memset(
                vtiles[i][:].rearrange("p (h c) -> p h c", h=2)[:, :, 64:65], 1.0
            )

        # ---------------- emission helpers ----------------
        from concourse.tile import add_dep_helper

        first_mm = [None]  # tile-0 last matmul, for const-DMA deferral
        xt0_dma = [None]

        def emit_qkv_tile(ti):
            xt = sb_x.tile([128, 8, 128], F16, tag="xt", name=f"xt{ti}")
            nc.sync.dma_start(out=xt[:, 0:4, :], in_=d_xtb[ti, :, 0:4, :])
            dma = nc.sync.dma_start(out=xt[:, 4:8, :], in_=d_xtb[ti, :, 4:8, :])
            if ti == 0:
                xt0_dma[0] = dma
            psq = ps.tile([128, 384], F32, tag="qkvtr", name=f"psq{ti}")
            for i in range(8):
                mm = nc.tensor.matmul(
                    psq[:], xt[:, i, :], wqkv[:, i, :],
                    start=(i == 0), stop=(i == 7),
                )
            if ti == 0:
                first_mm[0] = mm
                for cd in const_dmas:
                    add_dep_helper(cd.ins, mm.ins, True, "defer const DMA")
                for cd in early_dmas:
                    add_dep_helper(cd.ins, xt0_dma[0].ins, True, "defer early DMA")
            # stats: Square (ACT, reads PSUM) + segmented reduce (DVE)
            sqt = sb_w1.tile([128, 256], F16, tag="sqt", name=f"sqt{ti}")
            nc.scalar.activation(sqt[:], psq[:, 0:256], AF.Square)
            # single fast evacuation of the q|k halves (frees the bank)
            qsb = sb_w1.tile([128, 256], F32, tag="qsb", name=f"qsb{ti}")
            nc.scalar.copy(qsb[:], psq[:, 0:256])
            nc.vector.tensor_reduce(
                stats[:, 4 * ti : 4 * ti + 4],
                sqt[:].rearrange("p (s c) -> p s c", s=4),
                axis=AX.X, op=ALU.add,
            )
            # v-blend straight from PSUM (DVE), fp16 out
            vt = vtiles[ti]
            nc.vector.scalar_tensor_tensor(
                out=vt[:].rearrange("p (s c) -> p s c", s=2)[:, :, 0:64],
                in0=psq[:, 256:384].rearrange("p (s c) -> p s c", s=2),
                scalar=1.0,
                in1=vi_t[:, ti, :].rearrange("p (s c) -> p s c", s=2),
                op0=ALU.mult, op1=ALU.add,
            )
            # rope: swap absorbed into strided views; tsin pair + tcos on DVE,
            # add on Pool
            qk4 = qsb[:].rearrange("p (s h c) -> p s h c", s=4, h=2)
            tsin = sb_w2.tile([128, 256], F32, tag="tsin", name=f"tsin{ti}")
            t4 = tsin[:].rearrange("p (s h c) -> p s h c", s=4, h=2)
            nc.vector.tensor_tensor(
                out=t4[:, :, 0, :],
                in0=qk4[:, :, 1, :],
                in1=sc_t[:, ti, 0:32].unsqueeze(1).broadcast_to((128, 4, 32)),
                op=ALU.mult,
            )
            nc.vector.tensor_tensor(
                out=t4[:, :, 1, :],
                in0=qk4[:, :, 0, :],
                in1=sc_t[:, ti, 32:64].unsqueeze(1).broadcast_to((128, 4, 32)),
                op=ALU.mult,
            )
            tcos = sb_w2.tile([128, 256], F32, tag="tcos", name=f"tcos{ti}")
            nc.vector.tensor_tensor(
                out=tcos[:].rearrange("p (s c) -> p s c", s=4),
                in0=qsb[:].rearrange("p (s c) -> p s c", s=4),
                in1=cc_t[:, ti, :].unsqueeze(1).broadcast_to((128, 4, 64)),
                op=ALU.mult,
            )
            nc.gpsimd.tensor_tensor(
                out=qkro[ti][:], in0=tcos[:], in1=tsin[:], op=ALU.add
            )

        def emit_chain(g):
            # batched rsqrt for tiles 4g..4g+3 (DVE bit-trick + 1 Newton
            # iter; keeps ACT on the exp table only), then norm applies
            gg = 16 * g
            rs = rbuf[:, gg : gg + 16]
            zt = sb_w2.tile([128, 16], F32, tag="zt", name=f"zt{g}")
            nt1 = sb_w2.tile([128, 16], F32, tag="nt1", name=f"nt1{g}")
            nc.vector.tensor_scalar(
                out=zt[:], in0=stats[:, gg : gg + 16], scalar1=1.0 / 64.0,
                scalar2=RMS_EPS, op0=ALU.mult, op1=ALU.add,
            )
            nc.vector.tensor_scalar(
                out=nt1[:].bitcast(I32), in0=zt[:].bitcast(I32), scalar1=1,
                scalar2=0xFFFFFFFF, op0=ALU.logical_shift_right,
                op1=ALU.bitwise_xor,
            )
            nc.vector.tensor_scalar(
                out=rs.bitcast(I32), in0=nt1[:].bitcast(I32),
                scalar1=0x5F3759E0, scalar2=None, op0=ALU.add,
            )
            for _ in range(1):
                nc.vector.tensor_tensor(out=nt1[:], in0=rs, in1=rs, op=ALU.mult)
                nc.vector.tensor_tensor(out=nt1[:], in0=nt1[:], in1=zt[:], op=ALU.mult)
                nc.vector.tensor_scalar(
                    out=nt1[:], in0=nt1[:], scalar1=-0.5, scalar2=1.5,
                    op0=ALU.mult, op1=ALU.add,
                )
                nc.vector.tensor_tensor(out=rs, in0=rs, in1=nt1[:], op=ALU.mult)
            # fold 0.125 into the k columns of rbuf (cols 4t+2, 4t+3)
            kv = rbuf[:, gg : gg + 16].rearrange("p (t c) -> p t c", c=4)[:, :, 2:4]
            nc.vector.tensor_scalar_mul(kv, kv, 0.125)
            for tj in range(4 * g, 4 * g + 4):
                nc.vector.tensor_tensor(
                    out=qkr[tj][:].rearrange("p (s c) -> p s c", s=4),
                    in0=qkro[tj][:].rearrange("p (s c) -> p s c", s=4),
                    in1=rbuf[:, 4 * tj : 4 * tj + 4]
                    .unsqueeze(2)
                    .broadcast_to((128, 4, 64)),
                    op=ALU.mult,
                )

        def emit_transposes(g):
            for tj in range(4 * g, 4 * g + 4):
                for which, dst in ((0, qT), (1, kT)):
                    ptr = ps.tile(
                        [128, 128], F16, tag="qkvtr", name=f"tr{tj}_{which}"
                    )
                    nc.tensor.transpose(
                        ptr[:], qkr[tj][:, 128 * which : 128 * which + 128], idn[:]
                    )
                    nc.vector.tensor_copy(
                        dst[:, 128 * tj : 128 * (tj + 1)], ptr[:]
                    )

        def emit_attention(ci):
            # software-pipelined per (head, kj) units: S (score matmul) and
            # E (exp) run LAG units ahead of Y (P@V matmul) so the PE always
            # has score work while ACT/DVE chew on exp/tri of earlier units.
            LAG = 3
            kj_max = 4 * ci + 4
            yt_h = [
                ps.tile([128, 512], F32, tag="ytmo", name=f"yt{ci}_{h}")
                for h in range(2)
            ]
            units = []
            for kj in range(kj_max):
                for h in range(2):
                    units.append((h, kj))
            ets = {}

            def emit_SE(u):
                h, kj = u
                q0 = 128 * (kj - 4 * ci) if kj >= 4 * ci else 0
                st = ps3.tile([128, 512], F32, tag="st", name=f"st{ci}_{h}_{kj}")
                et = sb_e.tile([128, 512], F16, tag="et", name=f"et{ci}_{h}_{kj}")
                ets[u] = (et, q0)
                nc.tensor.matmul(
                    st[:, q0:512],
                    kT[64 * h : 64 * h + 64, 128 * kj : 128 * (kj + 1)],
                    qT[64 * h : 64 * h + 64, 512 * ci + q0 : 512 * (ci + 1)],
                    start=True, stop=True,
                )
                nc.scalar.activation(et[:, q0:512], st[:, q0:512], AF.Exp)
                if kj >= 4 * ci:  # diagonal: tri-mask the block
                    blk = et[:, q0 : q0 + 128]
                    eng = nc.vector if kj % 2 == 0 else nc.gpsimd
                    eng.tensor_tensor(out=blk, in0=blk, in1=tri[:], op=ALU.mult)

            def emit_Y(u):
                h, kj = u
                et, q0 = ets.pop(u)
                nc.tensor.matmul(
                    yt_h[h][0:65, q0:512],
                    vtiles[kj][:, 65 * h : 65 * h + 65],
                    et[:, q0:512],
                    start=(kj == 0), stop=(kj == kj_max - 1),
                )

            for i, u in enumerate(units):
                emit_SE(u)
                if i >= LAG:
                    emit_Y(units[i - LAG])
            for u in units[-LAG:]:
                emit_Y(u)
            return yt_h

        def emit_scale_outproj(ci, yt_h):
            for h in range(2):
                # L + e^sink on the [1,512] L-row, reciprocal, then a
                # partition-broadcast view feeds the yts scale directly
                lr = sb_w2.tile([1, 512], F32, tag="lr", name=f"lr{ci}_{h}")
                nc.vector.tensor_tensor(
                    out=lr[:],
                    in0=yt_h[h][64:65, 0:512],
                    in1=esk[0:1, h : h + 1].broadcast_to((1, 512)),
                    op=ALU.add,
                )
                rr = sb_w2.tile([1, 512], F32, tag="rr", name=f"rr{ci}_{h}")
                nc.vector.reciprocal_approx_fast(out=rr[:], in_=lr[:])
                rrb = sb_w2.tile([64, 512], F32, tag="rrb", name=f"rrb{ci}_{h}")
                nc.gpsimd.partition_broadcast(rrb[:], rr[:])
                if h == 0:
                    nc.vector.tensor_tensor(
                        out=yts[0:64, 512 * ci : 512 * (ci + 1)],
                        in0=yt_h[h][0:64, 0:512],
                        in1=rrb[:],
                        op=ALU.mult,
                    )
                else:
                    yts1 = sb_w2.tile([64, 512], F16, tag="yts1", name=f"yts1_{ci}")
                    nc.vector.tensor_tensor(
                        out=yts1[:],
                        in0=yt_h[h][0:64, 0:512],
                        in1=rrb[:],
                        op=ALU.mult,
                    )
                    nc.gpsimd.dma_start(
                        out=yts[64:128, 512 * ci : 512 * (ci + 1)], in_=yts1[:]
                    )
            for jt in range(8):
                pso = ps.tile([128, 512], F32, tag="ytmo", name=f"pso{ci}_{jt}")
                nc.tensor.matmul(
                    pso[:],
                    wo[:, 128 * jt : 128 * (jt + 1)],
                    yts[:, 512 * ci : 512 * (ci + 1)],
                    start=True, stop=True,
                )
                outsb = sb_o.tile([128, 512], F16, tag="outsb", name=f"osb{ci}_{jt}")
                if jt % 2 == 0:
                    nc.vector.tensor_copy(outsb[:], pso[:])
                else:
                    nc.scalar.copy(outsb[:], pso[:])
                q = nc.sync if jt % 2 == 0 else nc.gpsimd
                q.dma_start(
                    out=d_out[128 * jt : 128 * (jt + 1), 512 * ci : 512 * (ci + 1)],
                    in_=outsb[:],
                )

        # ---------------- HAM warm-up: ~4us of junk matmuls ----------------
        wz = sb.tile([128, 512], F16)
        nc.gpsimd.memset(wz[:], 0.0)
        pwz = ps.tile([128, 512], F32, tag="ytmo", name="pwz")
        for _w in range(6):
            nc.tensor.matmul(
                pwz[:], wz[:, 0:128], wz[:], start=True, stop=True
            )

        # ---------------- interleaved emission ----------------
        # PE stream per window ci: ATT(ci) | QKV(next 4) | OUT(ci) | T(ci+1)
        # so rope/rsqrt (DVE) for the next chunk overlaps this chunk's
        # attention/out-proj PE work.
        for ti in range(4):
            emit_qkv_tile(ti)
        emit_chain(0)
        emit_transposes(0)
        for ci in range(4):
            yt_h = emit_attention(ci)
            if ci < 3:
                for ti in range(4 * ci + 4, 4 * ci + 8):
                    emit_qkv_tile(ti)
                emit_chain(ci + 1)
            emit_scale_outproj(ci, yt_h)
            if ci < 3:
                emit_transposes(ci + 1)

    nc.compile()
    return nc


_NC = None


def _rope_tables():
    inv = (1.0 / 10000.0) ** (np.arange(0, HD, 2, dtype=np.float64) / HD)
    t = np.arange(T, dtype=np.float64)
    f = np.outer(t, inv)  # (T, 32)
    cc = np.concatenate([np.cos(f), np.cos(f)], axis=1).astype(np.float32)
    sc = np.concatenate([np.sin(f), -np.sin(f)], axis=1).astype(np.float32)
    return cc, sc


def kernel(x, vi, Wq, Wk, Wv, Wo, lamb, sink_weights):
    global _NC
    x = np.asarray(x, dtype=np.float32)
    vi = np.asarray(vi, dtype=np.float32)
    Wq = np.asarray(Wq, dtype=np.float32)
    Wk = np.asarray(Wk, dtype=np.float32)
    Wv = np.asarray(Wv, dtype=np.float32)
    Wo = np.asarray(Wo, dtype=np.float32)
    lam = float(np.asarray(lamb).reshape(-1)[0])
    sink = np.asarray(sink_weights, dtype=np.float32).reshape(-1)

    if _NC is None:
        _NC = _build_program()

    x0T = x[0].T  # (D, T)
    xtb = np.ascontiguousarray(
        x0T.reshape(8, 128, NT, 128).transpose(2, 1, 0, 3)
    ).astype(np.float16)  # (NT, p, i, c): xtb[ti, p, n, c] = xT[128n+p, 128ti+c]
    cc, sc = _rope_tables()
    ccb = np.ascontiguousarray(cc.reshape(NT, 128, 64).transpose(1, 0, 2))
    scb = np.ascontiguousarray(sc.reshape(NT, 128, 64).transpose(1, 0, 2))
    tri = (np.arange(128)[None, :] >= np.arange(128)[:, None]).astype(np.float16)
    idn = np.eye(128, dtype=np.float16)

    in_maps = []
    for c in range(8):
        lo = 128 * c
        wqkv = np.concatenate(
            [
                Wq[lo : lo + 128].T,
                Wk[lo : lo + 128].T,
                (1.0 - lam) * Wv[lo : lo + 128].T,
            ],
            axis=1,
        )  # (D, 384)
        wqkv = np.ascontiguousarray(
            wqkv.reshape(8, 128, 384).transpose(1, 0, 2)
        ).astype(np.float16)
        esk = np.exp(sink[2 * c : 2 * c + 2]).astype(np.float32).reshape(1, 2)
        in_maps.append(
            {
                "xtb": xtb,
                "wqkv": wqkv,
                "vis": np.ascontiguousarray(
                    (lam * vi[0][:, lo : lo + 128]).reshape(NT, 128, 128).transpose(1, 0, 2)
                ).astype(np.float16),
                "cc": ccb,
                "sc": scb,
                "wo": np.ascontiguousarray(Wo[:, lo : lo + 128].T).astype(np.float16),
                "idn": idn,
                "tri": tri,
                "esk": esk,
            }
        )

    global _trace_in_maps
    _trace_in_maps = in_maps
    res = None
    for attempt in range(3):
        try:
            res = run_bass_kernel_spmd(_NC, in_maps, list(range(8)))
            break
        except Exception:
            # transient NRT_EXEC_UNIT_UNRECOVERABLE flakes have been seen on
            # the first execute after a fresh compile; retry
            if attempt == 2:
                raise
    outT = np.zeros((D, T), np.float64)
    for c in range(8):
        outT += res.results[c]["out"].astype(np.float64)
    return np.ascontiguousarray(outT.T).astype(np.float32).reshape(1, T, D)


# revision 12
# speedup vs baseline: 1.4370x; 1.4370x over previous
"""Trainium2 Bass kernel for nn_CausalSelfAttention (B=1, T=2048, D=1024, H=16).

Sharding: 2 heads per core across 8 cores (tensor parallel). Wq/Wk/Wv
column-sharded by head, attention fully local, Wo row-sharded; host sums the
8 partial outputs (the all-reduce of the unshard step).

Per-core pipeline (all matmuls fp16 operands, fp32 PSUM accumulate):
  P1  fused QKV: psum[t,384] = sum_i xT_blk.T @ [WqT|WkT|(1-l)WvT]; wqkv DMA'd
      per-i so accumulation streams behind the loads. RMS stats (Square on ACT
      from PSUM + segmented reduce -> rsqrt bit-trick, 1 Newton iter, 0.125
      folded into k's scale), RoPE with the swap absorbed into strided views,
      v-blend (+lam*vi) via DVE scalar_tensor_tensor straight from PSUM,
      [vA|1|vB|1] fp16 tiles with ones columns memset once.
  P2  PE-transpose roped q,k (fp16 all the way, fp16 PSUM) -> qT,kT fp16.
  P3  attention software-pipelined per (head, kj) unit: score matmul into a
      1-bank PSUM tile, Exp (ACT) on just the live columns, tri-mask on
      diagonal blocks (DVE/Pool alternating), P@V matmul lagging 3 units so
      the PE never waits on Exp; lhsT=[v|1] accumulates L in row 64.
  P4  scale = 1/(L + e^sink): row-64 + e^sink (DVE) -> reciprocal ->
      partition-broadcast view into the yts scale mult; out-proj per 128-row
      block (K=128), evacuate fp16, DMA fp16 partials on alternating queues;
      host upcasts + sums.
"""

import sys

if "/opt/trn_rl_repo" not in sys.path:
    sys.path.insert(0, "/opt/trn_rl_repo")

import numpy as np
from contextlib import ExitStack

from concourse import bacc, tile
from concourse import mybir
from concourse.bass_utils import run_bass_kernel_spmd

F32 = mybir.dt.float32
F32R = mybir.dt.float32r
F16 = mybir.dt.float16
I32 = mybir.dt.int32
AF = mybir.ActivationFunctionType
ALU = mybir.AluOpType
AX = mybir.AxisListType

T = 2048
D = 1024
HD = 64
NT = T // 128  # 16 t-tiles
RMS_EPS = float(np.finfo(np.float32).eps)


def _build_program():
    nc = bacc.Bacc("TRN2", target_bir_lowering=False, debug=False, num_devices=8)

    d_xtb = nc.dram_tensor("xtb", [NT, 128, 8, 128], F16, kind="ExternalInput").ap()
    d_wqkv = nc.dram_tensor("wqkv", [128, 8, 384], F16, kind="ExternalInput").ap()
    d_vis = nc.dram_tensor("vis", [128, NT, 128], F16, kind="ExternalInput").ap()
    d_cc = nc.dram_tensor("cc", [128, NT, 64], F32, kind="ExternalInput").ap()
    d_sc = nc.dram_tensor("sc", [128, NT, 64], F32, kind="ExternalInput").ap()
    d_wo = nc.dram_tensor("wo", [128, D], F16, kind="ExternalInput").ap()
    d_idn = nc.dram_tensor("idn", [128, 128], F16, kind="ExternalInput").ap()
    d_tri = nc.dram_tensor("tri", [128, 128], F16, kind="ExternalInput").ap()
    d_esk = nc.dram_tensor("esk", [1, 2], F32, kind="ExternalInput").ap()
    d_onr = nc.dram_tensor("onr", [1, 64], F32R, kind="ExternalInput").ap()
    d_out = nc.dram_tensor("out", [D, T], F16, kind="ExternalOutput").ap()

    with tile.TileContext(nc) as tc, ExitStack() as ctx:
        sb = ctx.enter_context(tc.tile_pool(name="sb", bufs=1))
        sb_x = ctx.enter_context(tc.tile_pool(name="sb_x", bufs=5))
        sb_w1 = ctx.enter_context(tc.tile_pool(name="sb_w1", bufs=3))
        sb_w2 = ctx.enter_context(tc.tile_pool(name="sb_w2", bufs=3))
        sb_e = ctx.enter_context(tc.tile_pool(name="sb_e", bufs=6))
        sb_o = ctx.enter_context(tc.tile_pool(name="sb_o", bufs=3))
        ps = ctx.enter_context(tc.tile_pool(name="ps", bufs=2, space="PSUM"))
        ps3 = ctx.enter_context(tc.tile_pool(name="ps3", bufs=3, space="PSUM"))

        # weights first on the sync queue (needed by the first matmul),
        # split per-i so QKV accumulation can stream behind the loads;
        # other constants go via the gpsimd queue so they don't delay x.
        wqkv = sb.tile([128, 8, 384], F16)
        wq_dmas = []
        for _wi in range(8):
            wq_dmas.append(
                nc.sync.dma_start(
                    out=wqkv[:, _wi, :], in_=d_wqkv[:, _wi, :]
                )
            )
        vi_t = sb.tile([128, NT, 128], F16)
        cc_t = sb.tile([128, NT, 64], F32)
        sc_t = sb.tile([128, NT, 64], F32)
        wo = sb.tile([128, D], F16)
        esk = sb.tile([1, 2], F32)
        const_dmas = []
        early_dmas = []
        early_dmas.append(nc.gpsimd.dma_start(out=vi_t[:], in_=d_vis[:]))
        early_dmas.append(nc.gpsimd.dma_start(out=cc_t[:], in_=d_cc[:]))
        early_dmas.append(nc.gpsimd.dma_start(out=sc_t[:], in_=d_sc[:]))
        early_dmas.append(nc.gpsimd.dma_start(out=esk[:], in_=d_esk[:]))
        const_dmas.append(nc.gpsimd.dma_start(out=wo[:], in_=d_wo[:]))
        idn = sb.tile([128, 128], F16)
        early_dmas.append(nc.gpsimd.dma_start(out=idn[:], in_=d_idn[:]))
        tri = sb.tile([128, 128], F16)
        const_dmas.append(nc.gpsimd.dma_start(out=tri[:], in_=d_tri[:]))

        stats = sb.tile([128, 64], F32)
        rbuf = sb.tile([128, 64], F32)
        qT = sb.tile([128, T], F16)
        kT = sb.tile([128, T], F16)
        vtiles = [sb.tile([128, 130], F16, tag=f"v{i}", name=f"v{i}") for i in range(NT)]
        qkr = [sb.tile([128, 256], F16, tag=f"qkr{i}", name=f"qkr{i}") for i in range(NT)]
        qkro = [sb.tile([128, 256], F32, tag=f"qkro{i}", name=f"qkro{i}") for i in range(NT)]
        yts = sb.tile([128, T], F16)
        onesr = sb.tile([1, 64], F32R)
        early_dmas.append(nc.gpsimd.dma_start(out=onesr[:], in_=d_onr[:]))

        # ones columns of the [vA|1|vB|1] tiles: written once, up front
        for i in range(NT):
            nc.gpsimd.memset(
                vtiles[i][:].rearrange("p (h c) -> p h c", h=2)[:, :, 64:65], 1.0
            )

        # ---------------- emission helpers ----------------
        from concourse.tile import add_dep_helper

        first_mm = [None]  # tile-0 last matmul, for const-DMA deferral
        xt0_dma = [None]

        def emit_qkv_tile(ti):
            xt = sb_x.tile([128, 8, 128], F16, tag="xt", name=f"xt{ti}")
            nc.sync.dma_start(out=xt[:, 0:4, :], in_=d_xtb[ti, :, 0:4, :])
            dma = nc.sync.dma_start(out=xt[:, 4:8, :], in_=d_xtb[ti, :, 4:8, :])
            if ti == 0:
                xt0_dma[0] = dma
            psq = ps.tile([128, 384], F32, tag="qkvtr", name=f"psq{ti}")
            for i in range(8):
                mm = nc.tensor.matmul(
                    psq[:], xt[:, i, :], wqkv[:, i, :],
                    start=(i == 0), stop=(i == 7),
                )
            if ti == 0:
                first_mm[0] = mm
                for cd in const_dmas:
                    add_dep_helper(cd.ins, mm.ins, True, "defer const DMA")
                for cd in early_dmas:
                    add_dep_helper(cd.ins, xt0_dma[0].ins, True, "defer early DMA")
            # stats: Square (ACT, reads PSUM) + segmented reduce (DVE)
            sqt = sb_w1.tile([128, 256], F16, tag="sqt", name=f"sqt{ti}")
            nc.scalar.activation(sqt[:], psq[:, 0:256], AF.Square)
            # single fast evacuation of the q|k halves (frees the bank)
            qsb = sb_w1.tile([128, 256], F32, tag="qsb", name=f"qsb{ti}")
            nc.scalar.copy(qsb[:], psq[:, 0:256])
            nc.vector.tensor_reduce(
                stats[:, 4 * ti : 4 * ti + 4],
                sqt[:].rearrange("p (s c) -> p s c", s=4),
                axis=AX.X, op=ALU.add,
            )
            # v-blend straight from PSUM (DVE), fp16 out
            vt = vtiles[ti]
            nc.vector.scalar_tensor_tensor(
                out=vt[:].rearrange("p (s c) -> p s c", s=2)[:, :, 0:64],
                in0=psq[:, 256:384].rearrange("p (s c) -> p s c", s=2),
                scalar=1.0,
                in1=vi_t[:, ti, :].rearrange("p (s c) -> p s c", s=2),
                op0=ALU.mult, op1=ALU.add,
            )
            # rope: swap absorbed into strided views; tsin pair + tcos on DVE,
            # add on Pool
            qk4 = qsb[:].rearrange("p (s h c) -> p s h c", s=4, h=2)
            tsin = sb_w2.tile([128, 256], F32, tag="tsin", name=f"tsin{ti}")
            t4 = tsin[:].rearrange("p (s h c) -> p s h c", s=4, h=2)
            nc.vector.tensor_tensor(
                out=t4[:, :, 0, :],
                in0=qk4[:, :, 1, :],
                in1=sc_t[:, ti, 0:32].unsqueeze(1).broadcast_to((128, 4, 32)),
                op=ALU.mult,
            )
            nc.vector.tensor_tensor(
                out=t4[:, :, 1, :],
                in0=qk4[:, :, 0, :],
                in1=sc_t[:, ti, 32:64].unsqueeze(1).broadcast_to((128, 4, 32)),
                op=ALU.mult,
            )
            tcos = sb_w2.tile([128, 256], F32, tag="tcos", name=f"tcos{ti}")
            nc.vector.tensor_tensor(
                out=tcos[:].rearrange("p (s c) -> p s c", s=4),
                in0=qsb[:].rearrange("p (s c) -> p s c", s=4),
                in1=cc_t[:, ti, :].unsqueeze(1).broadcast_to((128, 4, 64)),
                op=ALU.mult,
            )
            nc.gpsimd.tensor_tensor(
                out=qkro[ti][:], in0=tcos[:], in1=tsin[:], op=ALU.add
            )

        def emit_chain(g):
            # batched rsqrt for tiles 4g..4g+3 (DVE bit-trick + 1 Newton
            # iter; keeps ACT on the exp table only), then norm applies
            gg = 16 * g
            rs = rbuf[:, gg : gg + 16]
            zt = sb_w2.tile([128, 16], F32, tag="zt", name=f"zt{g}")
            nt1 = sb_w2.tile([128, 16], F32, tag="nt1", name=f"nt1{g}")
            nc.vector.tensor_scalar(
                out=zt[:], in0=stats[:, gg : gg + 16], scalar1=1.0 / 64.0,
                scalar2=RMS_EPS, op0=ALU.mult, op1=ALU.add,
            )
            nc.vector.tensor_scalar(
                out=nt1[:].bitcast(I32), in0=zt[:].bitcast(I32), scalar1=1,
                scalar2=0xFFFFFFFF, op0=ALU.logical_shift_right,
                op1=ALU.bitwise_xor,
            )
            nc.vector.tensor_scalar(
                out=rs.bitcast(I32), in0=nt1[:].bitcast(I32),
                scalar1=0x5F3759E0, scalar2=None, op0=ALU.add,
            )
            for _ in range(1):
                nc.vector.tensor_tensor(out=nt1[:], in0=rs, in1=rs, op=ALU.mult)
                nc.vector.tensor_tensor(out=nt1[:], in0=nt1[:], in1=zt[:], op=ALU.mult)
                nc.vector.tensor_scalar(
                    out=nt1[:], in0=nt1[:], scalar1=-0.5, scalar2=1.5,
                    op0=ALU.mult, op1=ALU.add,
                )
                nc.vector.tensor_tensor(out=rs, in0=rs, in1=nt1[:], op=ALU.mult)
            # fold 0.125 into the k columns of rbuf (cols 4t+2, 4t+3)
            kv = rbuf[:, gg : gg + 16].rearrange("p (t c) -> p t c", c=4)[:, :, 2:4]
            nc.vector.tensor_scalar_mul(kv, kv, 0.125)
            for tj in range(4 * g, 4 * g + 4):
                nc.vector.tensor_tensor(
                    out=qkr[tj][:].rearrange("p (s c) -> p s c", s=4),
                    in0=qkro[tj][:].rearrange("p (s c) -> p s c", s=4),
                    in1=rbuf[:, 4 * tj : 4 * tj + 4]
                    .unsqueeze(2)
                    .broadcast_to((128, 4, 64)),
                    op=ALU.mult,
                )

        def emit_transposes(g):
            for tj in range(4 * g, 4 * g + 4):
                for which, dst in ((0, qT), (1, kT)):
                    ptr = ps.tile(
                        [128, 128], F16, tag="qkvtr", name=f"tr{tj}_{which}"
                    )
                    nc.tensor.transpose(
                        ptr[:], qkr[tj][:, 128 * which : 128 * which + 128], idn[:]
                    )
                    nc.vector.tensor_copy(
                        dst[:, 128 * tj : 128 * (tj + 1)], ptr[:]
                    )

        def emit_attention(ci):
            # software-pipelined per (head, kj) units: S (score matmul) and
            # E (exp) run LAG units ahead of Y (P@V matmul) so the PE always
            # has score work while ACT/DVE chew on exp/tri of earlier units.
            LAG = 3
            kj_max = 4 * ci + 4
            yt_h = [
                ps.tile([128, 512], F32, tag="ytmo", name=f"yt{ci}_{h}")
                for h in range(2)
            ]
            units = []
            for kj in range(kj_max):
                for h in range(2):
                    units.append((h, kj))
            ets = {}

            def emit_SE(u):
                h, kj = u
                q0 = 128 * (kj - 4 * ci) if kj >= 4 * ci else 0
                st = ps3.tile([128, 512], F32, tag="st", name=f"st{ci}_{h}_{kj}")
                et = sb_e.tile([128, 512], F16, tag="et", name=f"et{ci}_{h}_{kj}")
                ets[u] = (et, q0)
                nc.tensor.matmul(
                    st[:, q0:512],
                    kT[64 * h : 64 * h + 64, 128 * kj : 128 * (kj + 1)],
                    qT[64 * h : 64 * h + 64, 512 * ci + q0 : 512 * (ci + 1)],
                    start=True, stop=True,
                )
                nc.scalar.activation(et[:, q0:512], st[:, q0:512], AF.Exp)
                if kj >= 4 * ci:  # diagonal: tri-mask the block
                    blk = et[:, q0 : q0 + 128]
                    eng = nc.vector if kj % 2 == 0 else nc.gpsimd
                    eng.tensor_tensor(out=blk, in0=blk, in1=tri[:], op=ALU.mult)

            def emit_Y(u):
                h, kj = u
                et, q0 = ets.pop(u)
                nc.tensor.matmul(
                    yt_h[h][0:65, q0:512],
                    vtiles[kj][:, 65 * h : 65 * h + 65],
                    et[:, q0:512],
                    start=(kj == 0), stop=(kj == kj_max - 1),
                )

            for i, u in enumerate(units):
                emit_SE(u)
                if i >= LAG:
                    emit_Y(units[i - LAG])
            for u in units[-LAG:]:
                emit_Y(u)
            return yt_h

        def emit_scale_outproj(ci, yt_h):
            for h in range(2):
                # L + e^sink on the [1,512] L-row, reciprocal, then a
                # partition-broadcast view feeds the yts scale directly
                lr = sb_w2.tile([1, 512], F32R, tag="lr", name=f"lr{ci}_{h}")
                nc.vector.tensor_tensor(
                    out=lr[:],
                    in0=yt_h[h][64:65, 0:512],
                    in1=esk[0:1, h : h + 1].broadcast_to((1, 512)),
                    op=ALU.add,
                )
                # broadcast L+e^sink across 64 partitions via a K=1 matmul
                # (gpsimd partition_broadcast thrashes the Pool microcode
                # library), then reciprocal
                mbp = ps3.tile([64, 512], F32, tag="st", name=f"mbp{ci}_{h}")
                nc.tensor.matmul(mbp[:], onesr[:], lr[:], start=True, stop=True)
                mbs = sb_w2.tile([64, 512], F32, tag="mbs", name=f"mbs{ci}_{h}")
                nc.vector.reciprocal_approx_fast(out=mbs[:], in_=mbp[:])
                if h == 0:
                    nc.vector.tensor_tensor(
                        out=yts[0:64, 512 * ci : 512 * (ci + 1)],
                        in0=yt_h[h][0:64, 0:512],
                        in1=mbs[:],
                        op=ALU.mult,
                    )
                else:
                    yts1 = sb_w2.tile([64, 512], F16, tag="yts1", name=f"yts1_{ci}")
                    nc.vector.tensor_tensor(
                        out=yts1[:],
                        in0=yt_h[h][0:64, 0:512],
                        in1=mbs[:],
                        op=ALU.mult,
                    )
                    nc.gpsimd.dma_start(
                        out=yts[64:128, 512 * ci : 512 * (ci + 1)], in_=yts1[:]
                    )
            for jt in range(8):
                pso = ps.tile([128, 512], F32, tag="ytmo", name=f"pso{ci}_{jt}")
                nc.tensor.matmul(
                    pso[:],
                    wo[:, 128 * jt : 128 * (jt + 1)],
                    yts[:, 512 * ci : 512 * (ci + 1)],
                    start=True, stop=True,
                )
                outsb = sb_o.tile([128, 512], F16, tag="outsb", name=f"osb{ci}_{jt}")
                if jt % 2 == 0:
                    nc.vector.tensor_copy(outsb[:], pso[:])
                else:
                    nc.scalar.copy(outsb[:], pso[:])
                q = nc.sync if jt % 2 == 0 else nc.gpsimd
                q.dma_start(
                    out=d_out[128 * jt : 128 * (jt + 1), 512 * ci : 512 * (ci + 1)],
                    in_=outsb[:],
                )

        # ---------------- HAM warm-up: ~4us of junk matmuls ----------------
        wz = sb.tile([128, 512], F16)
        nc.gpsimd.memset(wz[:], 0.0)
        pwz = ps.tile([128, 512], F32, tag="ytmo", name="pwz")
        for _w in range(6):
            nc.tensor.matmul(
                pwz[:], wz[:, 0:128], wz[:], start=True, stop=True
            )

        # ---------------- interleaved emission ----------------
        # PE stream per window ci: ATT(ci) | QKV(next 4) | OUT(ci) | T(ci+1)
        # so rope/rsqrt (DVE) for the next chunk overlaps this chunk's
        # attention/out-proj PE work.
        for ti in range(4):
            emit_qkv_tile(ti)
        emit_chain(0)
        emit_transposes(0)
        for ci in range(4):
            yt_h = emit_attention(ci)
            if ci < 3:
                for ti in range(4 * ci + 4, 4 * ci + 8):
                    emit_qkv_tile(ti)
                emit_chain(ci + 1)
            emit_scale_outproj(ci, yt_h)
            if ci < 3:
                emit_transposes(ci + 1)

    nc.compile()
    return nc


_NC = None


def _rope_tables():
    inv = (1.0 / 10000.0) ** (np.arange(0, HD, 2, dtype=np.float64) / HD)
    t = np.arange(T, dtype=np.float64)
    f = np.outer(t, inv)  # (T, 32)
    cc = np.concatenate([np.cos(f), np.cos(f)], axis=1).astype(np.float32)
    sc = np.concatenate([np.sin(f), -np.sin(f)], axis=1).astype(np.float32)
    return cc, sc


def kernel(x, vi, Wq, Wk, Wv, Wo, lamb, sink_weights):
    global _NC
    x = np.asarray(x, dtype=np.float32)
    vi = np.asarray(vi, dtype=np.float32)
    Wq = np.asarray(Wq, dtype=np.float32)
    Wk = np.asarray(Wk, dtype=np.float32)
    Wv = np.asarray(Wv, dtype=np.float32)
    Wo = np.asarray(Wo, dtype=np.float32)
    lam = float(np.asarray(lamb).reshape(-1)[0])
    sink = np.asarray(sink_weights, dtype=np.float32).reshape(-1)

    if _NC is None:
        _NC = _build_program()

    x0T = x[0].T  # (D, T)
    xtb = np.ascontiguousarray(
        x0T.reshape(8, 128, NT, 128).transpose(2, 1, 0, 3)
    ).astype(np.float16)  # (NT, p, i, c): xtb[ti, p, n, c] = xT[128n+p, 128ti+c]
    cc, sc = _rope_tables()
    ccb = np.ascontiguousarray(cc.reshape(NT, 128, 64).transpose(1, 0, 2))
    scb = np.ascontiguousarray(sc.reshape(NT, 128, 64).transpose(1, 0, 2))
    tri = (np.arange(128)[None, :] >= np.arange(128)[:, None]).astype(np.float16)
    idn = np.eye(128, dtype=np.float16)

    in_maps = []
    for c in range(8):
        lo = 128 * c
        wqkv = np.concatenate(
            [
                Wq[lo : lo + 128].T,
                Wk[lo : lo + 128].T,
                (1.0 - lam) * Wv[lo : lo + 128].T,
            ],
            axis=1,
        )  # (D, 384)
        wqkv = np.ascontiguousarray(
            wqkv.reshape(8, 128, 384).transpose(1, 0, 2)
        ).astype(np.float16)
        esk = np.exp(sink[2 * c : 2 * c + 2]).astype(np.float32).reshape(1, 2)
        in_maps.append(
            {
                "xtb": xtb,
                "wqkv": wqkv,
                "vis": np.ascontiguousarray(
                    (lam * vi[0][:, lo : lo + 128]).reshape(NT, 128, 128).transpose(1, 0, 2)
                ).astype(np.float16),
                "cc": ccb,
                "sc": scb,
                "wo": np.ascontiguousarray(Wo[:, lo : lo + 128].T).astype(np.float16),
                "idn": idn,
                "tri": tri,
                "esk": esk,
                "onr": np.ones((1, 64), np.float32),
            }
        )

    global _trace_in_maps
    _trace_in_maps = in_maps
    res = None
    for attempt in range(3):
        try:
            res = run_bass_kernel_spmd(_NC, in_maps, list(range(8)))
            break
        except Exception:
            # transient NRT_EXEC_UNIT_UNRECOVERABLE flakes have been seen on
            # the first execute after a fresh compile; retry
            if attempt == 2:
                raise
    outT = np.zeros((D, T), np.float64)
    for c in range(8):
        outT += res.results[c]["out"].astype(np.float64)
    return np.ascontiguousarray(outT.T).astype(np.float32).reshape(1, T, D)


# revision 21
# speedup vs baseline: 1.4585x; 1.0150x over previous
"""Trainium2 Bass kernel for nn_CausalSelfAttention (B=1, T=2048, D=1024, H=16).

Sharding: 2 heads per core across 8 cores (tensor parallel). Wq/Wk/Wv
column-sharded by head, attention fully local, Wo row-sharded; host sums the
8 partial outputs (the all-reduce of the unshard step).

Per-core pipeline (all matmuls fp16 operands, fp32 PSUM accumulate):
  P1  fused QKV: psum[t,384] = sum_i xT_blk.T @ [WqT|WkT|(1-l)WvT]; wqkv DMA'd
      per-i so accumulation streams behind the loads. RMS stats (Square on ACT
      from PSUM + segmented reduce -> rsqrt bit-trick, 1 Newton iter, 0.125
      folded into k's scale), RoPE with the swap absorbed into strided views,
      v-blend (+lam*vi) via DVE scalar_tensor_tensor straight from PSUM,
      [vA|1|vB|1] fp16 tiles with ones columns memset once.
  P2  PE-transpose roped q,k (fp16 all the way, fp16 PSUM) -> qT,kT fp16.
  P3  attention software-pipelined per (head, kj) unit: score matmul into a
      1-bank PSUM tile, Exp (ACT) on just the live columns, tri-mask on
      diagonal blocks (DVE/Pool alternating), P@V matmul lagging 3 units so
      the PE never waits on Exp; lhsT=[v|1] accumulates L in row 64.
  P4  scale = 1/(L + e^sink): row-64 + e^sink (DVE) -> reciprocal ->
      partition-broadcast view into the yts scale mult; out-proj per 128-row
      block (K=128), evacuate fp16, DMA fp16 partials on alternating queues;
      host upcasts + sums.
"""

import sys

if "/opt/trn_rl_repo" not in sys.path:
    sys.path.insert(0, "/opt/trn_rl_repo")

import numpy as np
from contextlib import ExitStack

from concourse import bacc, tile
from concourse import mybir
from concourse.bass_utils import run_bass_kernel_spmd

F32 = mybir.dt.float32
F32R = mybir.dt.float32r
F16 = mybir.dt.float16
I32 = mybir.dt.int32
AF = mybir.ActivationFunctionType
ALU = mybir.AluOpType
AX = mybir.AxisListType

T = 2048
D = 1024
HD = 64
NT = T // 128  # 16 t-tiles
RMS_EPS = float(np.finfo(np.float32).eps)


def _build_program():
    nc = bacc.Bacc("TRN2", target_bir_lowering=False, debug=False, num_devices=8)

    d_xtb = nc.dram_tensor("xtb", [NT, 128, 8, 128], F16, kind="ExternalInput").ap()
    d_wqkv = nc.dram_tensor("wqkv", [128, 8, 384], F16, kind="ExternalInput").ap()
    d_vis = nc.dram_tensor("vis", [128, NT, 128], F16, kind="ExternalInput").ap()
    d_cc = nc.dram_tensor("cc", [128, NT, 64], F32, kind="ExternalInput").ap()
    d_sc = nc.dram_tensor("sc", [128, NT, 64], F32, kind="ExternalInput").ap()
    d_wo = nc.dram_tensor("wo", [128, D], F16, kind="ExternalInput").ap()
    d_idn = nc.dram_tensor("idn", [128, 128], F16, kind="ExternalInput").ap()
    d_tri = nc.dram_tensor("tri", [128, 128], F16, kind="ExternalInput").ap()
    d_esk = nc.dram_tensor("esk", [1, 2], F16, kind="ExternalInput").ap()
    d_onr = nc.dram_tensor("onr", [1, 64], F16, kind="ExternalInput").ap()
    d_out = nc.dram_tensor("out", [D, T], F16, kind="ExternalOutput").ap()

    with tile.TileContext(nc) as tc, ExitStack() as ctx:
        sb = ctx.enter_context(tc.tile_pool(name="sb", bufs=1))
        sb_x = ctx.enter_context(tc.tile_pool(name="sb_x", bufs=5))
        sb_w1 = ctx.enter_context(tc.tile_pool(name="sb_w1", bufs=3))
        sb_w2 = ctx.enter_context(tc.tile_pool(name="sb_w2", bufs=3))
        sb_e = ctx.enter_context(tc.tile_pool(name="sb_e", bufs=6))
        sb_o = ctx.enter_context(tc.tile_pool(name="sb_o", bufs=3))
        ps = ctx.enter_context(tc.tile_pool(name="ps", bufs=2, space="PSUM"))
        psq3 = ctx.enter_context(tc.tile_pool(name="psq3", bufs=3, space="PSUM"))
        ps3 = ctx.enter_context(tc.tile_pool(name="ps3", bufs=3, space="PSUM"))

        # weights first on the sync queue (needed by the first matmul),
        # split per-i so QKV accumulation can stream behind the loads;
        # other constants go via the gpsimd queue so they don't delay x.
        wqkv = sb.tile([128, 8, 384], F16)
        wq_dmas = []
        for _wi in range(8):
            wq_dmas.append(
                nc.sync.dma_start(
                    out=wqkv[:, _wi, :], in_=d_wqkv[:, _wi, :]
                )
            )
        vi_t = sb.tile([128, NT, 128], F16)
        cc_t = sb.tile([128, NT, 64], F32)
        sc_t = sb.tile([128, NT, 64], F32)
        wo = sb.tile([128, D], F16)
        esk = sb.tile([1, 2], F16)
        const_dmas = []
        early_dmas = []
        early_dmas.append(nc.gpsimd.dma_start(out=vi_t[:], in_=d_vis[:]))
        early_dmas.append(nc.gpsimd.dma_start(out=cc_t[:], in_=d_cc[:]))
        early_dmas.append(nc.gpsimd.dma_start(out=sc_t[:], in_=d_sc[:]))
        early_dmas.append(nc.gpsimd.dma_start(out=esk[:], in_=d_esk[:]))
        const_dmas.append(nc.gpsimd.dma_start(out=wo[:], in_=d_wo[:]))
        idn = sb.tile([128, 128], F16)
        early_dmas.append(nc.gpsimd.dma_start(out=idn[:], in_=d_idn[:]))
        tri = sb.tile([128, 128], F16)
        const_dmas.append(nc.gpsimd.dma_start(out=tri[:], in_=d_tri[:]))

        stats = sb.tile([128, 64], F32)
        rbuf = sb.tile([128, 64], F32)
        qT = sb.tile([128, T], F16)
        kT = sb.tile([128, T], F16)
        vtiles = [sb.tile([128, 130], F16, tag=f"v{i}", name=f"v{i}") for i in range(NT)]
        qkr = [sb.tile([128, 256], F16, tag=f"qkr{i}", name=f"qkr{i}") for i in range(NT)]
        qkro = [sb.tile([128, 256], F32, tag=f"qkro{i}", name=f"qkro{i}") for i in range(NT)]
        yts = sb.tile([128, T], F16)
        onesr = sb.tile([1, 64], F16)
        early_dmas.append(nc.gpsimd.dma_start(out=onesr[:], in_=d_onr[:]))

        # warm-up junk buffer first so the HAM warm-up matmuls start ASAP,
        # then the ones columns of the [vA|1|vB|1] tiles (written once)
        wz = sb.tile([128, 512], F16)
        nc.gpsimd.memset(wz[:], 0.0)
        for i in range(NT):
            nc.gpsimd.memset(
                vtiles[i][:].rearrange("p (h c) -> p h c", h=2)[:, :, 64:65], 1.0
            )

        # ---------------- emission helpers ----------------
        from concourse.tile import add_dep_helper

        first_mm = [None]  # tile-0 last matmul, for const-DMA deferral
        xt0_dma = [None]

        def emit_qkv_tile(ti):
            xt = sb_x.tile([128, 8, 128], F16, tag="xt", name=f"xt{ti}")
            nsp = 4 if ti < 4 else 2  # finer splits early -> more DMA rings
            w = 8 // nsp
            for sp in range(nsp):
                dma = nc.sync.dma_start(
                    out=xt[:, w * sp : w * (sp + 1), :],
                    in_=d_xtb[ti, :, w * sp : w * (sp + 1), :],
                )
            if ti == 0:
                xt0_dma[0] = dma
            psq = psq3.tile([128, 384], F32, tag="psq", name=f"psq{ti}")
            for i in range(8):
                mm = nc.tensor.matmul(
                    psq[:], xt[:, i, :], wqkv[:, i, :],
                    start=(i == 0), stop=(i == 7),
                )
            if ti == 0:
                first_mm[0] = mm
                for cd in const_dmas:
                    add_dep_helper(cd.ins, mm.ins, True, "defer const DMA")
                for cd in early_dmas:
                    add_dep_helper(cd.ins, xt0_dma[0].ins, True, "defer early DMA")
            # stats: Square (ACT, reads PSUM) + segmented reduce (DVE)
            sqt = sb_w1.tile([128, 256], F16, tag="sqt", name=f"sqt{ti}")
            nc.scalar.activation(sqt[:], psq[:, 0:256], AF.Square)
            nc.vector.tensor_reduce(
                stats[:, 4 * ti : 4 * ti + 4],
                sqt[:].rearrange("p (s c) -> p s c", s=4),
                axis=AX.X, op=ALU.add,
            )
            # v-blend straight from PSUM (DVE), fp16 out
            vt = vtiles[ti]
            nc.vector.scalar_tensor_tensor(
                out=vt[:].rearrange("p (s c) -> p s c", s=2)[:, :, 0:64],
                in0=psq[:, 256:384].rearrange("p (s c) -> p s c", s=2),
                scalar=1.0,
                in1=vi_t[:, ti, :].rearrange("p (s c) -> p s c", s=2),
                op0=ALU.mult, op1=ALU.add,
            )
            # rope straight from PSUM: swap absorbed into strided views;
            # tsin pair + tcos on DVE, add on Pool
            qk4 = psq[:, 0:256].rearrange("p (s h c) -> p s h c", s=4, h=2)
            tsin = sb_w2.tile([128, 256], F32, tag="tsin", name=f"tsin{ti}")
            t4 = tsin[:].rearrange("p (s h c) -> p s h c", s=4, h=2)
            nc.vector.tensor_tensor(
                out=t4[:, :, 0, :],
                in0=qk4[:, :, 1, :],
                in1=sc_t[:, ti, 0:32].unsqueeze(1).broadcast_to((128, 4, 32)),
                op=ALU.mult,
            )
            nc.vector.tensor_tensor(
                out=t4[:, :, 1, :],
                in0=qk4[:, :, 0, :],
                in1=sc_t[:, ti, 32:64].unsqueeze(1).broadcast_to((128, 4, 32)),
                op=ALU.mult,
            )
            tcos = sb_w2.tile([128, 256], F32, tag="tcos", name=f"tcos{ti}")
            nc.vector.tensor_tensor(
                out=tcos[:].rearrange("p (s c) -> p s c", s=4),
                in0=psq[:, 0:256].rearrange("p (s c) -> p s c", s=4),
                in1=cc_t[:, ti, :].unsqueeze(1).broadcast_to((128, 4, 64)),
                op=ALU.mult,
            )
            nc.gpsimd.tensor_tensor(
                out=qkro[ti][:], in0=tcos[:], in1=tsin[:], op=ALU.add
            )

        def emit_chain(g):
            # batched rsqrt for tiles 4g..4g+3 (DVE bit-trick + 1 Newton
            # iter; keeps ACT on the exp table only), then norm applies
            gg = 16 * g
            rs = rbuf[:, gg : gg + 16]
            zt = sb_w2.tile([128, 16], F32, tag="zt", name=f"zt{g}")
            nt1 = sb_w2.tile([128, 16], F32, tag="nt1", name=f"nt1{g}")
            nc.vector.tensor_scalar(
                out=zt[:], in0=stats[:, gg : gg + 16], scalar1=1.0 / 64.0,
                scalar2=RMS_EPS, op0=ALU.mult, op1=ALU.add,
            )
            nc.vector.tensor_scalar(
                out=nt1[:].bitcast(I32), in0=zt[:].bitcast(I32), scalar1=1,
                scalar2=0xFFFFFFFF, op0=ALU.logical_shift_right,
                op1=ALU.bitwise_xor,
            )
            nc.vector.tensor_scalar(
                out=rs.bitcast(I32), in0=nt1[:].bitcast(I32),
                scalar1=0x5F3759E0, scalar2=None, op0=ALU.add,
            )
            for _ in range(1):
                nc.vector.tensor_tensor(out=nt1[:], in0=rs, in1=rs, op=ALU.mult)
                nc.vector.tensor_tensor(out=nt1[:], in0=nt1[:], in1=zt[:], op=ALU.mult)
                nc.vector.tensor_scalar(
                    out=nt1[:], in0=nt1[:], scalar1=-0.5, scalar2=1.5,
                    op0=ALU.mult, op1=ALU.add,
                )
                nc.vector.tensor_tensor(out=rs, in0=rs, in1=nt1[:], op=ALU.mult)
            # fold 0.125 into the k columns of rbuf (cols 4t+2, 4t+3)
            kv = rbuf[:, gg : gg + 16].rearrange("p (t c) -> p t c", c=4)[:, :, 2:4]
            nc.vector.tensor_scalar_mul(kv, kv, 0.125)
            for tj in range(4 * g, 4 * g + 4):
                nc.vector.tensor_tensor(
                    out=qkr[tj][:].rearrange("p (s c) -> p s c", s=4),
                    in0=qkro[tj][:].rearrange("p (s c) -> p s c", s=4),
                    in1=rbuf[:, 4 * tj : 4 * tj + 4]
                    .unsqueeze(2)
                    .broadcast_to((128, 4, 64)),
                    op=ALU.mult,
                )

        def emit_transposes(g):
            for tj in range(4 * g, 4 * g + 4):
                for which, dst in ((0, qT), (1, kT)):
                    ptr = ps3.tile(
                        [128, 128], F16, tag="st", name=f"tr{tj}_{which}"
                    )
                    nc.tensor.transpose(
                        ptr[:], qkr[tj][:, 128 * which : 128 * which + 128], idn[:]
                    )
                    nc.vector.tensor_copy(
                        dst[:, 128 * tj : 128 * (tj + 1)], ptr[:]
                    )

        def emit_attention(ci):
            # software-pipelined per (head, kj) units: S (score matmul) and
            # E (exp) run LAG units ahead of Y (P@V matmul) so the PE always
            # has score work while ACT/DVE chew on exp/tri of earlier units.
            LAG = 3
            kj_max = 4 * ci + 4
            yt_h = [
                ps.tile([128, 512], F32, tag="ytmo", name=f"yt{ci}_{h}")
                for h in range(2)
            ]
            units = []
            for kj in range(kj_max):
                for h in range(2):
                    units.append((h, kj))
            ets = {}

            def emit_SE(u):
                h, kj = u
                q0 = 128 * (kj - 4 * ci) if kj >= 4 * ci else 0
                st = ps3.tile([128, 512], F32, tag="st", name=f"st{ci}_{h}_{kj}")
                et = sb_e.tile([128, 512], F16, tag="et", name=f"et{ci}_{h}_{kj}")
                ets[u] = (et, q0)
                nc.tensor.matmul(
                    st[:, q0:512],
                    kT[64 * h : 64 * h + 64, 128 * kj : 128 * (kj + 1)],
                    qT[64 * h : 64 * h + 64, 512 * ci + q0 : 512 * (ci + 1)],
                    start=True, stop=True,
                )
                nc.scalar.activation(et[:, q0:512], st[:, q0:512], AF.Exp)
                if kj >= 4 * ci:  # diagonal: tri-mask the block
                    blk = et[:, q0 : q0 + 128]
                    eng = nc.vector if kj % 2 == 0 else nc.gpsimd
                    eng.tensor_tensor(out=blk, in0=blk, in1=tri[:], op=ALU.mult)

            def emit_Y(u):
                h, kj = u
                et, q0 = ets.pop(u)
                nc.tensor.matmul(
                    yt_h[h][0:65, q0:512],
                    vtiles[kj][:, 65 * h : 65 * h + 65],
                    et[:, q0:512],
                    start=(kj == 0), stop=(kj == kj_max - 1),
                )

            for i, u in enumerate(units):
                emit_SE(u)
                if i >= LAG:
                    emit_Y(units[i - LAG])
            for u in units[-LAG:]:
                emit_Y(u)
            return yt_h

        def emit_scale_outproj(ci, yt_h):
            for h in range(2):
                # (L + e^sink)/16 in fp16 (scale keeps fp16 in range),
                # broadcast across 64 partitions via a K=1 fp16 matmul
                # (gpsimd partition_broadcast thrashes the Pool microcode
                # library), reciprocal, then scale with the 1/16 folded in
                lr = sb_w2.tile([1, 512], F16, tag="lr", name=f"lr{ci}_{h}")
                nc.vector.scalar_tensor_tensor(
                    out=lr[:],
                    in0=yt_h[h][64:65, 0:512],
                    scalar=0.0625,
                    in1=esk[0:1, h : h + 1].broadcast_to((1, 512)),
                    op0=ALU.mult, op1=ALU.add,
                )
                mbp = ps3.tile([64, 512], F32, tag="st", name=f"mbp{ci}_{h}")
                nc.tensor.matmul(mbp[:], onesr[:], lr[:], start=True, stop=True)
                mbs = sb_w2.tile([64, 512], F32, tag="mbs", name=f"mbs{ci}_{h}")
                nc.vector.reciprocal_approx_fast(out=mbs[:], in_=mbp[:])
                if h == 0:
                    nc.vector.scalar_tensor_tensor(
                        out=yts[0:64, 512 * ci : 512 * (ci + 1)],
                        in0=yt_h[h][0:64, 0:512],
                        scalar=0.0625,
                        in1=mbs[:],
                        op0=ALU.mult, op1=ALU.mult,
                    )
                else:
                    yts1 = sb_w2.tile([64, 512], F16, tag="yts1", name=f"yts1_{ci}")
                    nc.vector.scalar_tensor_tensor(
                        out=yts1[:],
                        in0=yt_h[h][0:64, 0:512],
                        scalar=0.0625,
                        in1=mbs[:],
                        op0=ALU.mult, op1=ALU.mult,
                    )
                    nc.gpsimd.dma_start(
                        out=yts[64:128, 512 * ci : 512 * (ci + 1)], in_=yts1[:]
                    )
            for jt in range(8):
                pso = ps.tile([128, 512], F32, tag="ytmo", name=f"pso{ci}_{jt}")
                nc.tensor.matmul(
                    pso[:],
                    wo[:, 128 * jt : 128 * (jt + 1)],
                    yts[:, 512 * ci : 512 * (ci + 1)],
                    start=True, stop=True,
                )
                outsb = sb_o.tile([128, 512], F16, tag="outsb", name=f"osb{ci}_{jt}")
                if jt % 2 == 0:
                    nc.vector.tensor_copy(outsb[:], pso[:])
                else:
                    nc.scalar.copy(outsb[:], pso[:])
                q = nc.sync if jt % 2 == 0 else nc.gpsimd
                q.dma_start(
                    out=d_out[128 * jt : 128 * (jt + 1), 512 * ci : 512 * (ci + 1)],
                    in_=outsb[:],
                )

        # ---------------- HAM warm-up: ~4us of junk matmuls ----------------
        pwz = ps.tile([128, 512], F32, tag="ytmo", name="pwz")
        for _w in range(6):
            nc.tensor.matmul(
                pwz[:], wz[:, 0:128], wz[:], start=True, stop=True
            )

        # ---------------- interleaved emission ----------------
        # PE stream per window ci: ATT(ci) | QKV(next 4) | T(ci+1) | OUT(ci)
        # so rope/rsqrt (DVE) for the next chunk overlaps this chunk's
        # attention PE work, and T(ci+1) covers the lr/recip latency before
        # the out-proj matmuls.
        for ti in range(4):
            emit_qkv_tile(ti)
        emit_chain(0)
        emit_transposes(0)
        for ci in range(4):
            yt_h = emit_attention(ci)
            if ci < 3:
                for ti in range(4 * ci + 4, 4 * ci + 8):
                    emit_qkv_tile(ti)
                emit_chain(ci + 1)
                emit_transposes(ci + 1)
            emit_scale_outproj(ci, yt_h)

    nc.compile()
    return nc


_NC = None


def _rope_tables():
    inv = (1.0 / 10000.0) ** (np.arange(0, HD, 2, dtype=np.float64) / HD)
    t = np.arange(T, dtype=np.float64)
    f = np.outer(t, inv)  # (T, 32)
    cc = np.concatenate([np.cos(f), np.cos(f)], axis=1).astype(np.float32)
    sc = np.concatenate([np.sin(f), -np.sin(f)], axis=1).astype(np.float32)
    return cc, sc


def kernel(x, vi, Wq, Wk, Wv, Wo, lamb, sink_weights):
    global _NC
    x = np.asarray(x, dtype=np.float32)
    vi = np.asarray(vi, dtype=np.float32)
    Wq = np.asarray(Wq, dtype=np.float32)
    Wk = np.asarray(Wk, dtype=np.float32)
    Wv = np.asarray(Wv, dtype=np.float32)
    Wo = np.asarray(Wo, dtype=np.float32)
    lam = float(np.asarray(lamb).reshape(-1)[0])
    sink = np.asarray(sink_weights, dtype=np.float32).reshape(-1)

    if _NC is None:
        _NC = _build_program()

    x0T = x[0].T  # (D, T)
    xtb = np.ascontiguousarray(
        x0T.reshape(8, 128, NT, 128).transpose(2, 1, 0, 3)
    ).astype(np.float16)  # (NT, p, i, c): xtb[ti, p, n, c] = xT[128n+p, 128ti+c]
    cc, sc = _rope_tables()
    ccb = np.ascontiguousarray(cc.reshape(NT, 128, 64).transpose(1, 0, 2))
    scb = np.ascontiguousarray(sc.reshape(NT, 128, 64).transpose(1, 0, 2))
    tri = (np.arange(128)[None, :] >= np.arange(128)[:, None]).astype(np.float16)
    idn = np.eye(128, dtype=np.float16)

    in_maps = []
    for c in range(8):
        lo = 128 * c
        wqkv = np.concatenate(
            [
                Wq[lo : lo + 128].T,
                Wk[lo : lo + 128].T,
                (1.0 - lam) * Wv[lo : lo + 128].T,
            ],
            axis=1,
        )  # (D, 384)
        wqkv = np.ascontiguousarray(
            wqkv.reshape(8, 128, 384).transpose(1, 0, 2)
        ).astype(np.float16)
        esk = (np.exp(sink[2 * c : 2 * c + 2]) / 16.0).astype(np.float16).reshape(1, 2)
        in_maps.append(
            {
                "xtb": xtb,
                "wqkv": wqkv,
                "vis": np.ascontiguousarray(
                    (lam * vi[0][:, lo : lo + 128]).reshape(NT, 128, 128).transpose(1, 0, 2)
                ).astype(np.float16),
                "cc": ccb,
                "sc": scb,
                "wo": np.ascontiguousarray(Wo[:, lo : lo + 128].T).astype(np.float16),
                "idn": idn,
                "tri": tri,
                "esk": esk,
                "onr": np.ones((1, 64), np.float16),
            }
        )

    global _trace_in_maps
    _trace_in_maps = in_maps
    res = None
    for attempt in range(3):
        try:
            res = run_bass_kernel_spmd(_NC, in_maps, list(range(8)))
            break
        except Exception:
            # transient NRT_EXEC_UNIT_UNRECOVERABLE flakes have been seen on
            # the first execute after a fresh compile; retry
            if attempt == 2:
                raise
    outT = np.zeros((D, T), np.float64)
    for c in range(8):
        outT += res.results[c]["out"].astype(np.float64)
    return np.ascontiguousarray(outT.T).astype(np.float32).reshape(1, T, D)


# revision 24
# speedup vs baseline: 1.6416x; 1.1256x over previous
"""Trainium2 Bass kernel for nn_CausalSelfAttention (B=1, T=2048, D=1024, H=16).

Sharding: 2 heads per core across 8 cores (tensor parallel). Wq/Wk/Wv
column-sharded by head, attention fully local, Wo row-sharded; host sums the
8 partial outputs (the all-reduce of the unshard step).

Per-core pipeline (all matmuls fp16 operands, fp32 PSUM accumulate):
  P1  fused QKV: psum[t,384] = sum_i xT_blk.T @ [WqT|WkT|(1-l)WvT]; wqkv DMA'd
      per-i so accumulation streams behind the loads. RMS stats (Square on ACT
      from PSUM + segmented reduce -> rsqrt bit-trick, 1 Newton iter, 0.125
      folded into k's scale), RoPE with the swap absorbed into strided views,
      v-blend (+lam*vi) via DVE scalar_tensor_tensor straight from PSUM,
      [vA|1|vB|1] fp16 tiles with ones columns memset once.
  P2  PE-transpose roped q,k (fp16 all the way, fp16 PSUM) -> qT,kT fp16.
  P3  attention software-pipelined per (head, kj) unit: score matmul into a
      1-bank PSUM tile, Exp (ACT) on just the live columns, tri-mask on
      diagonal blocks (DVE/Pool alternating), P@V matmul lagging 3 units so
      the PE never waits on Exp; lhsT=[v|1] accumulates L in row 64.
  P4  scale = 1/(L + e^sink): row-64 + e^sink (DVE) -> reciprocal ->
      partition-broadcast view into the yts scale mult; out-proj per 128-row
      block (K=128), evacuate fp16, DMA fp16 partials on alternating queues;
      host upcasts + sums.
"""

import sys

if "/opt/trn_rl_repo" not in sys.path:
    sys.path.insert(0, "/opt/trn_rl_repo")

import numpy as np
from contextlib import ExitStack

from concourse import bacc, tile
from concourse import mybir
from concourse.bass_utils import run_bass_kernel_spmd

F32 = mybir.dt.float32
F32R = mybir.dt.float32r
F16 = mybir.dt.float16
I32 = mybir.dt.int32
AF = mybir.ActivationFunctionType
ALU = mybir.AluOpType
AX = mybir.AxisListType

T = 2048
D = 1024
HD = 64
NT = T // 128  # 16 t-tiles
RMS_EPS = float(np.finfo(np.float32).eps)


def _build_program():
    nc = bacc.Bacc("TRN2", target_bir_lowering=False, debug=False, num_devices=8)

    d_xtb = nc.dram_tensor("xtb", [NT, 128, 8, 128], F16, kind="ExternalInput").ap()
    d_wqkv = nc.dram_tensor("wqkv", [128, 8, 384], F16, kind="ExternalInput").ap()
    d_vis = nc.dram_tensor("vis", [128, NT, 128], F16, kind="ExternalInput").ap()
    d_cc = nc.dram_tensor("cc", [128, NT, 64], F32, kind="ExternalInput").ap()
    d_sc = nc.dram_tensor("sc", [128, NT, 64], F32, kind="ExternalInput").ap()
    d_wo = nc.dram_tensor("wo", [128, D], F16, kind="ExternalInput").ap()
    d_idn = nc.dram_tensor("idn", [128, 128], F16, kind="ExternalInput").ap()
    d_tri = nc.dram_tensor("tri", [128, 128], F16, kind="ExternalInput").ap()
    d_esk = nc.dram_tensor("esk", [1, 2], F16, kind="ExternalInput").ap()
    d_onr = nc.dram_tensor("onr", [1, 64], F16, kind="ExternalInput").ap()
    d_out = nc.dram_tensor("out", [D, T], F16, kind="ExternalOutput").ap()

    with tile.TileContext(nc) as tc, ExitStack() as ctx:
        sb = ctx.enter_context(tc.tile_pool(name="sb", bufs=1))
        sb_x = ctx.enter_context(tc.tile_pool(name="sb_x", bufs=5))
        sb_w1 = ctx.enter_context(tc.tile_pool(name="sb_w1", bufs=3))
        sb_w2 = ctx.enter_context(tc.tile_pool(name="sb_w2", bufs=3))
        sb_e = ctx.enter_context(tc.tile_pool(name="sb_e", bufs=4))
        sb_o = ctx.enter_context(tc.tile_pool(name="sb_o", bufs=3))
        ps = ctx.enter_context(tc.tile_pool(name="ps", bufs=2, space="PSUM"))
        psq3 = ctx.enter_context(tc.tile_pool(name="psq3", bufs=2, space="PSUM"))
        ps3 = ctx.enter_context(tc.tile_pool(name="ps3", bufs=2, space="PSUM"))

        # weights first on the sync queue (needed by the first matmul),
        # split per-i so QKV accumulation can stream behind the loads;
        # other constants go via the gpsimd queue so they don't delay x.
        wqkv = sb.tile([128, 8, 384], F16)
        wq_dmas = []
        for _wi in range(8):
            wq_dmas.append(
                nc.sync.dma_start(
                    out=wqkv[:, _wi, :], in_=d_wqkv[:, _wi, :]
                )
            )
        vi_t = sb.tile([128, NT, 128], F16)
        cc_t = sb.tile([128, NT, 64], F32)
        sc_t = sb.tile([128, NT, 64], F32)
        wo = sb.tile([128, D], F16)
        esk = sb.tile([1, 2], F16)
        const_dmas = []
        early_dmas = []
        early_dmas.append(nc.gpsimd.dma_start(out=vi_t[:], in_=d_vis[:]))
        early_dmas.append(nc.gpsimd.dma_start(out=cc_t[:], in_=d_cc[:]))
        early_dmas.append(nc.gpsimd.dma_start(out=sc_t[:], in_=d_sc[:]))
        early_dmas.append(nc.gpsimd.dma_start(out=esk[:], in_=d_esk[:]))
        const_dmas.append(nc.gpsimd.dma_start(out=wo[:], in_=d_wo[:]))
        idn = sb.tile([128, 128], F16)
        early_dmas.append(nc.gpsimd.dma_start(out=idn[:], in_=d_idn[:]))
        tri = sb.tile([128, 128], F16)
        const_dmas.append(nc.gpsimd.dma_start(out=tri[:], in_=d_tri[:]))

        stats = sb.tile([128, 64], F32)
        rbuf = sb.tile([128, 64], F32)
        qT = sb.tile([128, T], F16)
        kT = sb.tile([128, T], F16)
        vtiles = [sb.tile([128, 130], F16, tag=f"v{i}", name=f"v{i}") for i in range(NT)]
        qkr = [sb.tile([128, 256], F16, tag=f"qkr{i}", name=f"qkr{i}") for i in range(NT)]
        qkro = [sb.tile([128, 256], F32, tag=f"qkro{i}", name=f"qkro{i}") for i in range(NT)]
        yts = sb.tile([128, T], F16)
        onesr = sb.tile([1, 64], F16)
        early_dmas.append(nc.gpsimd.dma_start(out=onesr[:], in_=d_onr[:]))

        # warm-up junk buffer first so the HAM warm-up matmuls start ASAP,
        # then the ones columns of the [vA|1|vB|1] tiles (written once)
        wz = sb.tile([128, 512], F16)
        nc.gpsimd.memset(wz[:], 0.0)
        for i in range(NT):
            nc.gpsimd.memset(
                vtiles[i][:].rearrange("p (h c) -> p h c", h=2)[:, :, 64:65], 1.0
            )

        # ---------------- emission helpers ----------------
        from concourse.tile import add_dep_helper

        first_mm = [None]  # tile-0 last matmul, for const-DMA deferral
        xt0_dma = [None]

        def emit_qkv_tile(ti):
            xt = sb_x.tile([128, 8, 128], F16, tag="xt", name=f"xt{ti}")
            nsp = 4 if ti < 4 else 2  # finer splits early -> more DMA rings
            w = 8 // nsp
            for sp in range(nsp):
                dma = nc.sync.dma_start(
                    out=xt[:, w * sp : w * (sp + 1), :],
                    in_=d_xtb[ti, :, w * sp : w * (sp + 1), :],
                )
            if ti == 0:
                xt0_dma[0] = dma
            psq = psq3.tile([128, 384], F32, tag="psq", name=f"psq{ti}")
            for i in range(8):
                mm = nc.tensor.matmul(
                    psq[:], xt[:, i, :], wqkv[:, i, :],
                    start=(i == 0), stop=(i == 7),
                )
            if ti == 0:
                first_mm[0] = mm
                for cd in const_dmas:
                    add_dep_helper(cd.ins, mm.ins, True, "defer const DMA")
                for cd in early_dmas:
                    add_dep_helper(cd.ins, xt0_dma[0].ins, True, "defer early DMA")
            # stats: Square (ACT, reads PSUM) + segmented reduce (DVE)
            sqt = sb_w1.tile([128, 256], F16, tag="sqt", name=f"sqt{ti}")
            nc.scalar.activation(sqt[:], psq[:, 0:256], AF.Square)
            nc.vector.tensor_reduce(
                stats[:, 4 * ti : 4 * ti + 4],
                sqt[:].rearrange("p (s c) -> p s c", s=4),
                axis=AX.X, op=ALU.add,
            )
            # v-blend straight from PSUM (DVE), fp16 out
            vt = vtiles[ti]
            nc.vector.scalar_tensor_tensor(
                out=vt[:].rearrange("p (s c) -> p s c", s=2)[:, :, 0:64],
                in0=psq[:, 256:384].rearrange("p (s c) -> p s c", s=2),
                scalar=1.0,
                in1=vi_t[:, ti, :].rearrange("p (s c) -> p s c", s=2),
                op0=ALU.mult, op1=ALU.add,
            )
            # rope straight from PSUM: swap absorbed into strided views;
            # tsin pair + tcos on DVE, add on Pool
            qk4 = psq[:, 0:256].rearrange("p (s h c) -> p s h c", s=4, h=2)
            tsin = sb_w2.tile([128, 256], F32, tag="tsin", name=f"tsin{ti}")
            t4 = tsin[:].rearrange("p (s h c) -> p s h c", s=4, h=2)
            nc.vector.tensor_tensor(
                out=t4[:, :, 0, :],
                in0=qk4[:, :, 1, :],
                in1=sc_t[:, ti, 0:32].unsqueeze(1).broadcast_to((128, 4, 32)),
                op=ALU.mult,
            )
            nc.vector.tensor_tensor(
                out=t4[:, :, 1, :],
                in0=qk4[:, :, 0, :],
                in1=sc_t[:, ti, 32:64].unsqueeze(1).broadcast_to((128, 4, 32)),
                op=ALU.mult,
            )
            tcos = sb_w2.tile([128, 256], F32, tag="tcos", name=f"tcos{ti}")
            nc.vector.tensor_tensor(
                out=tcos[:].rearrange("p (s c) -> p s c", s=4),
                in0=psq[:, 0:256].rearrange("p (s c) -> p s c", s=4),
                in1=cc_t[:, ti, :].unsqueeze(1).broadcast_to((128, 4, 64)),
                op=ALU.mult,
            )
            nc.gpsimd.tensor_tensor(
                out=qkro[ti][:], in0=tcos[:], in1=tsin[:], op=ALU.add
            )

        def emit_chain(g):
            # batched rsqrt for tiles 4g..4g+3 (DVE bit-trick + 1 Newton
            # iter; keeps ACT on the exp table only), then norm applies
            gg = 16 * g
            rs = rbuf[:, gg : gg + 16]
            zt = sb_w2.tile([128, 16], F32, tag="zt", name=f"zt{g}")
            nt1 = sb_w2.tile([128, 16], F32, tag="nt1", name=f"nt1{g}")
            nc.vector.tensor_scalar(
                out=zt[:], in0=stats[:, gg : gg + 16], scalar1=1.0 / 64.0,
                scalar2=RMS_EPS, op0=ALU.mult, op1=ALU.add,
            )
            nc.vector.tensor_scalar(
                out=nt1[:].bitcast(I32), in0=zt[:].bitcast(I32), scalar1=1,
                scalar2=0xFFFFFFFF, op0=ALU.logical_shift_right,
                op1=ALU.bitwise_xor,
            )
            nc.vector.tensor_scalar(
                out=rs.bitcast(I32), in0=nt1[:].bitcast(I32),
                scalar1=0x5F3759E0, scalar2=None, op0=ALU.add,
            )
            for _ in range(1):
                nc.vector.tensor_tensor(out=nt1[:], in0=rs, in1=rs, op=ALU.mult)
                nc.vector.tensor_tensor(out=nt1[:], in0=nt1[:], in1=zt[:], op=ALU.mult)
                nc.vector.tensor_scalar(
                    out=nt1[:], in0=nt1[:], scalar1=-0.5, scalar2=1.5,
                    op0=ALU.mult, op1=ALU.add,
                )
                nc.vector.tensor_tensor(out=rs, in0=rs, in1=nt1[:], op=ALU.mult)
            # fold 0.125 into the k columns of rbuf (cols 4t+2, 4t+3)
            kv = rbuf[:, gg : gg + 16].rearrange("p (t c) -> p t c", c=4)[:, :, 2:4]
            nc.vector.tensor_scalar_mul(kv, kv, 0.125)
            for tj in range(4 * g, 4 * g + 4):
                nc.vector.tensor_tensor(
                    out=qkr[tj][:].rearrange("p (s c) -> p s c", s=4),
                    in0=qkro[tj][:].rearrange("p (s c) -> p s c", s=4),
                    in1=rbuf[:, 4 * tj : 4 * tj + 4]
                    .unsqueeze(2)
                    .broadcast_to((128, 4, 64)),
                    op=ALU.mult,
                )

        def emit_transposes(g):
            for tj in range(4 * g, 4 * g + 4):
                for which, dst in ((0, qT), (1, kT)):
                    ptr = ps3.tile(
                        [128, 128], F16, tag="st", name=f"tr{tj}_{which}"
                    )
                    nc.tensor.transpose(
                        ptr[:], qkr[tj][:, 128 * which : 128 * which + 128], idn[:]
                    )
                    nc.vector.tensor_copy(
                        dst[:, 128 * tj : 128 * (tj + 1)], ptr[:]
                    )

        def emit_attention(ci, qkv_tiles=()):
            # software-pipelined per (head, duo-of-kj) units: S (2 score
            # matmuls into a 2-bank duo) and E (one exp per duo) run one unit
            # ahead of Y (2 accumulating P@V matmuls), and QKV tiles for the
            # next chunk are interleaved into the stream so the PE sees long
            # accumulation groups (which let the clock ramp to full).
            LAG = 1
            kj_max = 4 * ci + 4
            yt_h = [
                ps.tile([128, 512], F32, tag="ytmo", name=f"yt{ci}_{h}")
                for h in range(2)
            ]
            units = []
            for dd in range(kj_max // 2):
                for h in range(2):
                    units.append((h, dd))
            ets = {}

            def emit_SE(u):
                h, dd = u
                st = ps3.tile([128, 1024], F32, tag="st", name=f"st{ci}_{h}_{dd}")
                et = sb_e.tile([128, 1024], F16, tag="et", name=f"et{ci}_{h}_{dd}")
                qs = {}
                for j2 in range(2):
                    kj = 2 * dd + j2
                    qs[j2] = 128 * (kj - 4 * ci) if kj >= 4 * ci else 0
                    nc.tensor.matmul(
                        st[:, 512 * j2 + qs[j2] : 512 * (j2 + 1)],
                        kT[64 * h : 64 * h + 64, 128 * kj : 128 * (kj + 1)],
                        qT[64 * h : 64 * h + 64, 512 * ci + qs[j2] : 512 * (ci + 1)],
                        start=True, stop=True,
                    )
                ets[u] = (et, qs)
                if qs[0] >= 256:
                    # far-diagonal duo: exp only the useful columns
                    nc.scalar.activation(
                        et[:, qs[0] : 512], st[:, qs[0] : 512], AF.Exp
                    )
                    nc.scalar.activation(
                        et[:, 512 + qs[1] : 1024], st[:, 512 + qs[1] : 1024], AF.Exp
                    )
                else:
                    nc.scalar.activation(et[:], st[:], AF.Exp)
                for j2 in range(2):
                    kj = 2 * dd + j2
                    if kj >= 4 * ci:  # diagonal: tri-mask the block
                        blk = et[:, 512 * j2 + qs[j2] : 512 * j2 + qs[j2] + 128]
                        eng = nc.vector if kj % 2 == 0 else nc.gpsimd
                        eng.tensor_tensor(out=blk, in0=blk, in1=tri[:], op=ALU.mult)

            def emit_Y(u):
                h, dd = u
                et, qs = ets.pop(u)
                for j2 in range(2):
                    kj = 2 * dd + j2
                    nc.tensor.matmul(
                        yt_h[h][0:65, qs[j2] : 512],
                        vtiles[kj][:, 65 * h : 65 * h + 65],
                        et[:, 512 * j2 + qs[j2] : 512 * (j2 + 1)],
                        start=(kj == 0), stop=(kj == kj_max - 1 and j2 == 1),
                    )

            pending = list(qkv_tiles)
            n = len(units)
            qpos = set()
            if pending:
                step = max(1, n // len(pending))
                qpos = {min(n - 1, 1 + k * step) for k in range(len(pending))}
            for i, u in enumerate(units):
                emit_SE(u)
                if i >= LAG:
                    emit_Y(units[i - LAG])
                if pending and i in qpos:
                    emit_qkv_tile(pending.pop(0))
            for u in units[-LAG:]:
                emit_Y(u)
            for ti in pending:
                emit_qkv_tile(ti)
            return yt_h

        def emit_scale_outproj(ci, yt_h):
            for h in range(2):
                # (L + e^sink)/16 in fp16 (scale keeps fp16 in range),
                # broadcast across 64 partitions via a K=1 fp16 matmul
                # (gpsimd partition_broadcast thrashes the Pool microcode
                # library), reciprocal, then scale with the 1/16 folded in
                lr = sb_w2.tile([1, 512], F16, tag="lr", name=f"lr{ci}_{h}")
                nc.vector.scalar_tensor_tensor(
                    out=lr[:],
                    in0=yt_h[h][64:65, 0:512],
                    scalar=0.0625,
                    in1=esk[0:1, h : h + 1].broadcast_to((1, 512)),
                    op0=ALU.mult, op1=ALU.add,
                )
                mbp = ps3.tile([64, 512], F32, tag="st", name=f"mbp{ci}_{h}")
                nc.tensor.matmul(mbp[:], onesr[:], lr[:], start=True, stop=True)
                mbs = sb_w2.tile([64, 512], F32, tag="mbs", name=f"mbs{ci}_{h}")
                nc.vector.reciprocal_approx_fast(out=mbs[:], in_=mbp[:])
                if h == 0:
                    nc.vector.scalar_tensor_tensor(
                        out=yts[0:64, 512 * ci : 512 * (ci + 1)],
                        in0=yt_h[h][0:64, 0:512],
                        scalar=0.0625,
                        in1=mbs[:],
                        op0=ALU.mult, op1=ALU.mult,
                    )
                else:
                    yts1 = sb_w2.tile([64, 512], F16, tag="yts1", name=f"yts1_{ci}")
                    nc.vector.scalar_tensor_tensor(
                        out=yts1[:],
                        in0=yt_h[h][0:64, 0:512],
                        scalar=0.0625,
                        in1=mbs[:],
                        op0=ALU.mult, op1=ALU.mult,
                    )
                    nc.gpsimd.dma_start(
                        out=yts[64:128, 512 * ci : 512 * (ci + 1)], in_=yts1[:]
                    )
            for jt in range(8):
                pso = ps.tile([128, 512], F32, tag="ytmo", name=f"pso{ci}_{jt}")
                nc.tensor.matmul(
                    pso[:],
                    wo[:, 128 * jt : 128 * (jt + 1)],
                    yts[:, 512 * ci : 512 * (ci + 1)],
                    start=True, stop=True,
                )
                outsb = sb_o.tile([128, 512], F16, tag="outsb", name=f"osb{ci}_{jt}")
                if jt % 2 == 0:
                    nc.vector.tensor_copy(outsb[:], pso[:])
                else:
                    nc.scalar.copy(outsb[:], pso[:])
                q = nc.sync if jt % 2 == 0 else nc.gpsimd
                q.dma_start(
                    out=d_out[128 * jt : 128 * (jt + 1), 512 * ci : 512 * (ci + 1)],
                    in_=outsb[:],
                )

        # ---------------- HAM warm-up: ~4us of junk matmuls ----------------
        pwz = ps.tile([128, 512], F32, tag="ytmo", name="pwz")
        for _w in range(6):
            nc.tensor.matmul(
                pwz[:], wz[:, 0:128], wz[:], start=True, stop=True
            )

        # ---------------- interleaved emission ----------------
        # PE stream per window ci: ATT(ci) | QKV(next 4) | T(ci+1) | OUT(ci)
        # so rope/rsqrt (DVE) for the next chunk overlaps this chunk's
        # attention PE work, and T(ci+1) covers the lr/recip latency before
        # the out-proj matmuls.
        for ti in range(4):
            emit_qkv_tile(ti)
        emit_chain(0)
        emit_transposes(0)
        for ci in range(4):
            qkv_tiles = range(4 * ci + 4, 4 * ci + 8) if ci < 3 else ()
            yt_h = emit_attention(ci, qkv_tiles)
            if ci < 3:
                emit_chain(ci + 1)
                emit_transposes(ci + 1)
            emit_scale_outproj(ci, yt_h)

    nc.compile()
    return nc


_NC = None


def _rope_tables():
    inv = (1.0 / 10000.0) ** (np.arange(0, HD, 2, dtype=np.float64) / HD)
    t = np.arange(T, dtype=np.float64)
    f = np.outer(t, inv)  # (T, 32)
    cc = np.concatenate([np.cos(f), np.cos(f)], axis=1).astype(np.float32)
    sc = np.concatenate([np.sin(f), -np.sin(f)], axis=1).astype(np.float32)
    return cc, sc


def kernel(x, vi, Wq, Wk, Wv, Wo, lamb, sink_weights):
    global _NC
    x = np.asarray(x, dtype=np.float32)
    vi = np.asarray(vi, dtype=np.float32)
    Wq = np.asarray(Wq, dtype=np.float32)
    Wk = np.asarray(Wk, dtype=np.float32)
    Wv = np.asarray(Wv, dtype=np.float32)
    Wo = np.asarray(Wo, dtype=np.float32)
    lam = float(np.asarray(lamb).reshape(-1)[0])
    sink = np.asarray(sink_weights, dtype=np.float32).reshape(-1)

    if _NC is None:
        _NC = _build_program()

    x0T = x[0].T  # (D, T)
    xtb = np.ascontiguousarray(
        x0T.reshape(8, 128, NT, 128).transpose(2, 1, 0, 3)
    ).astype(np.float16)  # (NT, p, i, c): xtb[ti, p, n, c] = xT[128n+p, 128ti+c]
    cc, sc = _rope_tables()
    ccb = np.ascontiguousarray(cc.reshape(NT, 128, 64).transpose(1, 0, 2))
    scb = np.ascontiguousarray(sc.reshape(NT, 128, 64).transpose(1, 0, 2))
    tri = (np.arange(128)[None, :] >= np.arange(128)[:, None]).astype(np.float16)
    idn = np.eye(128, dtype=np.float16)

    in_maps = []
    for c in range(8):
        lo = 128 * c
        wqkv = np.concatenate(
            [
                Wq[lo : lo + 128].T,
                Wk[lo : lo + 128].T,
                (1.0 - lam) * Wv[lo : lo + 128].T,
            ],
            axis=1,
        )  # (D, 384)
        wqkv = np.ascontiguousarray(
            wqkv.reshape(8, 128, 384).transpose(1, 0, 2)
        ).astype(np.float16)
        esk = (np.exp(sink[2 * c : 2 * c + 2]) / 16.0).astype(np.float16).reshape(1, 2)
        in_maps.append(
            {
                "xtb": xtb,
                "wqkv": wqkv,
                "vis": np.ascontiguousarray(
                    (lam * vi[0][:, lo : lo + 128]).reshape(NT, 128, 128).transpose(1, 0, 2)
                ).astype(np.float16),
                "cc": ccb,
                "sc": scb,
                "wo": np.ascontiguousarray(Wo[:, lo : lo + 128].T).astype(np.float16),
                "idn": idn,
                "tri": tri,
                "esk": esk,
                "onr": np.ones((1, 64), np.float16),
            }
        )

    global _trace_in_maps
    _trace_in_maps = in_maps
    res = None
    for attempt in range(3):
        try:
            res = run_bass_kernel_spmd(_NC, in_maps, list(range(8)))
            break
        except Exception:
            # transient NRT_EXEC_UNIT_UNRECOVERABLE flakes have been seen on
            # the first execute after a fresh compile; retry
            if attempt == 2:
                raise
    outT = np.zeros((D, T), np.float64)
    for c in range(8):
        outT += res.results[c]["out"].astype(np.float64)
    return np.ascontiguousarray(outT.T).astype(np.float32).reshape(1, T, D)
